# revision 2
# baseline (speedup 1.0000x reference)
"""Trainium2 Bass kernel for nn_Attention_15908558865595.

Math: qk[b,h,s,:] is constant along the softmax axis (query is expanded
along it), and jax.nn.softmax subtracts the row max, so the attention
weights are exactly uniform (1/F). The output is therefore
    out[b,h,s,f] = mean(value[b,h,:,0])
broadcast over [S,F] -- independent of query/key. The kernel computes the
per-(b,h) mean on device and broadcast-writes the 128 MiB output at the
HBM-write roofline. Sharding: batch*heads (32 pairs) split 4-per-core
across 8 NeuronCores; no cross-device communication.

Per-core layout: the 16 MiB output is one flat [128, 32768] region --
partition p holds slab p//32 (slabs are contiguous and SLAB == 32*32768),
so a single fill tile where partition p carries mean(slab p//32) feeds
every output DMA.

Device program (raw bass Block, no TileContext -- drops the tile
entry/exit barrier rounds):
  scalar: 16 KB input DMA (hoisted pre-barrier into main, overlapping the
          runtime's engine-start window)
  DVE:    block-diagonal 1/F mask via disjoint memsets (hoisted
          pre-barrier; disjoint because DVE has no same-engine WAW
          interlock) -> reduce_sum to bf16 partials -> after the matmul,
          one tiny PSUM->SBUF bc copy (self-semaphore before the first
          fill read: no same-engine RAW interlock either) -> chunked
          broadcast fills [0:512], [512:2048]
  PE:     single-pass bf16 matmul mask @ partials -> bc[p] =
          mean(slab p//32) on every partition
  sync:   output DMAs: 512-col lead, 1536-col bridge, then 8 KB-descriptor
          bulk (2048x15 reps via stride-0 source loops); one completion
          semaphore, final wait keeps the program end after the last HBM
          write receipt.

Measured (quiet device): ~53.2 us = ~4.5 us head (input receipt ~2.1 us
post-issue + ~1.9 us compute/issue/first-byte chain) + ~40.4 us streaming
16.78 MB at ~415 GB/s avg (421 sustained, 94-97% of the 435 GB/s fabric
ceiling; 2 KB descriptors cost ~0.2 us/MiB vs 8 KB, >=15 KB collapse) +
~8.2 us runtime epilogue (unconditional per-execution model-switch
wrapper: ~250 serial semaphore-clear instructions split across engines,
gated on the last DMA receipt; measured floor 14.4 us for an empty
kernel -- not reachable from BIR).
"""
import sys

if "/opt/trn_rl_repo" not in sys.path:
    sys.path.insert(0, "/opt/trn_rl_repo")

import numpy as np

B, H, S, F = 2, 16, 1024, 1024
N_CORES = 8
BH = B * H
BH_PER_CORE = BH // N_CORES      # 4
P = 128
VCOLS = BH_PER_CORE * F // P     # 32 value elements per partition
SLAB = S * F                     # one (b,h) output slab
YPP = BH_PER_CORE * SLAB // P    # 131072 output elements per partition
GROUP = P // BH_PER_CORE         # 32 partitions per slab

_NC = None


def _build():
    import concourse.bacc as bacc
    import concourse.bass as bass
    from concourse import mybir

    nc = bacc.Bacc("TRN2", target_bir_lowering=False, debug=False, num_devices=N_CORES)

    vg_ap = nc.dram_tensor("vg", [P, VCOLS], mybir.dt.float32, kind="ExternalInput").ap()
    out_ap = nc.dram_tensor(
        "out", [BH_PER_CORE * SLAB], mybir.dt.float32, kind="ExternalOutput"
    ).ap()

    FB = 2048  # bulk fill cols (8 KB descriptors)

    with (
        nc.sbuf_tensor([P, VCOLS], mybir.dt.float32) as vgtile,
        nc.sbuf_tensor([P, P], mybir.dt.bfloat16) as gmask,
        nc.sbuf_tensor([P, 1], mybir.dt.bfloat16) as partials,
        nc.psum_tensor([P, 1], mybir.dt.float32) as bc_psum,
        nc.sbuf_tensor([P, 1], mybir.dt.float32) as bc_s,
        nc.sbuf_tensor([P, FB], mybir.dt.float32) as fill,
        nc.semaphore() as s_in,
        nc.semaphore() as s_g,
        nc.semaphore() as s_red,
        nc.semaphore() as s_mm,
        nc.semaphore() as s_f1,
        nc.semaphore() as s_f2,
        nc.semaphore() as s_bc,
        nc.semaphore() as sd,
        nc.Block() as block,
    ):
        @block.scalar
        def _(scalar):
            # hoisted into main pre-barrier by _hoist_preamble
            scalar.dma_start(vgtile[:], vg_ap[:]).then_inc(s_in, 16)

        @block.vector
        def _(vector):
            # G mask: block-diagonal [128,128], 1/F where p//32 == k//32.
            # All memsets write DISJOINT regions: DVE pipelines back-to-back
            # instructions with no interlock, so an overlapping zero-fill
            # racing the block writes would be a same-engine WAW hazard.
            # No input dependency — hoisted pre-barrier.
            last = None
            for g in range(BH_PER_CORE):
                r0, r1 = g * GROUP, (g + 1) * GROUP
                if r0 > 0:
                    vector.memset(gmask[r0:r1, 0:r0], 0.0)
                if r1 < P:
                    vector.memset(gmask[r0:r1, r1:P], 0.0)
                last = vector.memset(gmask[r0:r1, r0:r1], 1.0 / F)
            last.then_inc(s_g, 1)
            # input-dependent chain
            vector.wait_ge(s_in, 16)
            with nc.allow_low_precision(
                reason="bf16 partials: 2^-9 rel err, tolerance is 2e-2"
            ):
                vector.reduce_sum(
                    partials[:], vgtile[:], axis=mybir.AxisListType.X
                ).then_inc(s_red, 1)
            # PSUM reads halve DVE copy rate: one tiny PSUM->SBUF copy,
            # then broadcast fills from SBUF at full rate. The s_bc self-wait
            # forces the bc_s write to retire before fill1 reads it (DVE has
            # no same-engine RAW interlock).
            vector.wait_ge(s_mm, 1)
            vector.tensor_copy(out=bc_s[:], in_=bc_psum[:]).then_inc(s_bc, 1)
            vector.wait_ge(s_bc, 1)
            vector.tensor_copy(
                out=fill[:, 0:512], in_=bc_s[:, 0:1].to_broadcast((P, 512))
            ).then_inc(s_f1, 1)
            vector.tensor_copy(
                out=fill[:, 512:2048], in_=bc_s[:, 0:1].to_broadcast((P, 1536))
            ).then_inc(s_f2, 1)

        @block.tensor
        def _(tensor):
            tensor.wait_ge(s_g, 1)
            tensor.wait_ge(s_red, 1)
            nc.tensor.matmul(
                bc_psum[:], gmask[:], partials[:], start=True, stop=True
            ).then_inc(s_mm, 1)

        @block.sync
        def _(sync):
            flat = out_ap.rearrange("(p y) -> p y", p=P)

            def emit(start, cols, reps, fill_lo, fill_hi, wait_sem):
                sync.wait_ge(wait_sem, 1)
                dst = flat[:, start : start + reps * cols].rearrange(
                    "p (r x) -> p r x", x=cols
                )
                src = fill[:, fill_lo:fill_hi][:, None, :].to_broadcast(
                    (P, reps, cols)
                )
                sync.dma_start(dst, src).then_inc(sd, 16)

            n_dma = 0
            emit(0, 512, 1, 0, 512, s_f1)
            n_dma += 1
            emit(512, 1536, 1, 512, 2048, s_f2)
            n_dma += 1
            # bulk: 15 reps of 2048 in chunks of 8/7 — no sub-8KB tail
            start = 2048
            for reps in (8, 7):
                emit(start, FB, reps, 0, FB, s_f2)
                start += reps * FB
                n_dma += 1
            assert start == YPP, (start, YPP)
            sync.wait_ge(sd, 16 * n_dma)

    nc.compile()
    _hoist_preamble(nc)
    return nc


def _hoist_preamble(nc):
    """Move the input DMA (Activation) and the gmask memsets (DVE) from
    their block bbs into main, ahead of each engine's entry-barrier drain.
    They carry no waits and only touch our own buffers, so running them
    during the runtime's engine-start-skew window is safe and hides their
    latency under the barrier.
    """
    from concourse import mybir

    try:
        f = nc.m.functions[0]
        main_bb = f.blocks[0]

        def hoist(engine, want, max_n):
            moved = []
            for bb in f.blocks[1:]:
                for inst in list(bb.instructions):
                    if len(moved) >= max_n:
                        break
                    if want(inst) and inst.engine == engine:
                        if inst.sync_info and inst.sync_info.on_wait:
                            return
                        bb.instructions.remove(inst)
                        moved.append(inst)
                if moved:
                    break
            if not moved:
                return
            idx = next(
                k
                for k, i in enumerate(main_bb.instructions)
                if isinstance(i, mybir.InstDrain) and i.engine == engine
            )
            for j, inst in enumerate(moved):
                main_bb.instructions.insert(idx + j, inst)

        hoist(
            mybir.EngineType.Activation,
            lambda i: isinstance(i, mybir.InstDMACopy),
            max_n=1,
        )
        hoist(
            mybir.EngineType.DVE,
            lambda i: isinstance(i, mybir.InstMemset),
            max_n=3 * BH_PER_CORE - 2,
        )
    except (StopIteration, IndexError, AttributeError):
        pass


def _get_nc():
    global _NC
    if _NC is None:
        _NC = _build()
    return _NC


def run_device(value_flat: np.ndarray, **spmd_kwargs):
    """value_flat: [BH, F] f32. Returns (out [BH, S, F], BassKernelResults)."""
    from concourse.bass_utils import run_bass_kernel_spmd

    nc = _get_nc()
    in_maps = [
        {
            "vg": np.ascontiguousarray(
                value_flat[c * BH_PER_CORE : (c + 1) * BH_PER_CORE].reshape(P, VCOLS)
            )
        }
        for c in range(N_CORES)
    ]
    res = run_bass_kernel_spmd(nc, in_maps, list(range(N_CORES)), **spmd_kwargs)
    out = np.empty((BH, S, F), dtype=np.float32)
    for c in range(N_CORES):
        out[c * BH_PER_CORE : (c + 1) * BH_PER_CORE] = res.results[c]["out"].reshape(
            BH_PER_CORE, S, F
        )
    return out, res


def kernel(query: np.ndarray, key: np.ndarray, value: np.ndarray) -> np.ndarray:
    value_flat = np.ascontiguousarray(
        np.asarray(value, dtype=np.float32).reshape(BH, F)
    )
    out, _ = run_device(value_flat)
    return out.reshape(B, H, S, F)
